# revision 5
# baseline (speedup 1.0000x reference)
"""Multi-head causal attention with RoPE on 8 Trainium2 NeuronCores.

Problem: B=2, S=2048, D=1024, H=16 heads (dk=64), fp32, causal mask,
RoPE on Q/K, y = softmax(QK^T/sqrt(dk)) V projected by Wo.

Sharding: head-parallel. Core c owns 2 heads (columns c*128:(c+1)*128 of
the QKV projection output). Each core:
  1. computes Q^T,K^T,V for its heads from the full x (K-dim 1024 matmuls),
  2. applies RoPE in the transposed [head_dim, token] layout,
  3. runs causal attention with scores materialized transposed (ST[k,q]) so
     softmax needs no transposes: exp on ScalarE straight out of PSUM, the
     PV matmul consumes the exp'd tile as the moving operand, and a ones
     column appended to V makes the same matmul emit the softmax denominator,
  4. AllToAll (2 MB) flips head-sharded -> token-sharded,
  5. computes 1/8 of the output projection; host concatenates row slices.
A tiny dummy AllToAll at kernel start absorbs the one-time collective
warmup cost under the projection phase.
"""

import sys

for p in ("/opt/trn_rl_repo", "/root/.axon_site/_ro/trn_rl_repo"):
    if p not in sys.path:
        sys.path.insert(0, p)

import math

import numpy as np

import concourse.bass as bass
import concourse.tile as tile
from concourse import mybir
from concourse.bass_utils import run_bass_kernel_spmd

N_CORES = 8
B, S, D, H = 2, 2048, 1024, 16
DK = D // H          # 64
HPC = H // N_CORES   # heads per core = 2
FW = HPC * DK        # head-group width per core = 128
T = B * S            # 4096 flattened tokens
TCH = 512            # token chunk for projections
NCH = T // TCH       # 8 chunks
KT = 128             # k tile
QC = 512             # q chunk in attention
TSL = T // N_CORES   # 512 output rows per core

F32 = mybir.dt.float32


def _spill_waits(nc, max_other=1):
    """walrus in this container allows 1 sync-wait per instruction; move
    excess waits onto preceding single-wait NoOps on the same engine."""
    n_new = 0
    for bb in nc.m.functions[0].blocks:
        newlist = []
        changed = False
        for inst in bb.instructions:
            si = inst.sync_info
            if si is not None and si.on_wait and len(si.on_wait) > max_other:
                waits = list(si.on_wait)
                overflow, keep = waits[:-max_other], waits[-max_other:]
                while overflow:
                    chunk, overflow = overflow[:1], overflow[1:]
                    nop = mybir.InstNoOp(
                        name=f"waitspill{n_new}-{inst.name}", ins=[], outs=[]
                    )
                    nop.engine = inst.engine
                    nop.debug = inst.debug
                    nop.sync_info = mybir.SyncInfo(on_wait=chunk, on_update=[])
                    newlist.append(nop)
                    n_new += 1
                si.on_wait = keep
                inst.sync_info = si
                changed = True
            newlist.append(inst)
        if changed:
            bb.instructions = newlist
    return n_new


def build_kernel():
    nc = bass.Bass("TRN2", num_devices=N_CORES)

    xT = nc.dram_tensor("xT", [D, T], F32, kind="ExternalInput")
    wq = nc.dram_tensor("wq", [D, FW], F32, kind="ExternalInput")  # pre-scaled 1/sqrt(dk)
    wk = nc.dram_tensor("wk", [D, FW], F32, kind="ExternalInput")
    wv = nc.dram_tensor("wv", [D, FW], F32, kind="ExternalInput")
    woT = nc.dram_tensor("woT", [D, D], F32, kind="ExternalInput")
    ctab = nc.dram_tensor("ctab", [FW, S], F32, kind="ExternalInput")
    stab = nc.dram_tensor("stab", [FW, S], F32, kind="ExternalInput")
    masks = nc.dram_tensor("masks", [KT, 896], F32, kind="ExternalInput")
    y = nc.dram_tensor("y", [TSL, D], F32, kind="ExternalOutput")

    xT_r = xT.rearrange("(dt p) t -> p dt t", p=128)  # [128, 8, T]

    with tile.TileContext(nc) as tc:
        with (
            tc.tile_pool(name="const", bufs=1) as const,
            tc.tile_pool(name="xch", bufs=2) as xch,
            tc.tile_pool(name="qk", bufs=1) as qkpool,
            tc.tile_pool(name="tmp", bufs=3) as tmp,
            tc.tile_pool(name="pts", bufs=4) as pts,
            tc.tile_pool(name="lpool", bufs=2) as lpool,
            tc.tile_pool(name="wo", bufs=8) as wopool,
            tc.tile_pool(name="yout", bufs=2) as ypool,
            tc.tile_pool(name="mm", bufs=2, space="PSUM") as mmps,
            tc.tile_pool(name="st", bufs=3, space="PSUM") as stps,
            tc.tile_pool(name="pv", bufs=2, space="PSUM") as pvps,
            tc.tile_pool(name="dram", bufs=1, space="DRAM") as dram,
        ):
            # ---- collective warmup (hidden under projection phase) ----
            warm_in = dram.tile([8, 16], F32)
            warm_out = dram.tile([8, 16], F32)
            wtile = const.tile([1, 128], F32)
            nc.vector.memset(wtile, 0.0)
            nc.gpsimd.dma_start(out=warm_in[:, :], in_=wtile[:1, :128].rearrange("p (a f) -> (p a) f", a=8))
            nc.gpsimd.collective_compute(
                "AllToAll",
                mybir.AluOpType.bypass,
                replica_groups=[list(range(N_CORES))],
                ins=[warm_in[:].opt()],
                outs=[warm_out[:].opt()],
            )

            # ---- constants ----
            wq_sb = const.tile([128, 8, FW], F32)
            wk_sb = const.tile([128, 8, FW], F32)
            wv_sb = const.tile([128, 8, FW], F32)
            nc.sync.dma_start(out=wq_sb, in_=wq.rearrange("(dt p) f -> p dt f", p=128))
            nc.sync.dma_start(out=wk_sb, in_=wk.rearrange("(dt p) f -> p dt f", p=128))
            nc.sync.dma_start(out=wv_sb, in_=wv.rearrange("(dt p) f -> p dt f", p=128))
            c_sb = const.tile([FW, S], F32)
            s_sb = const.tile([FW, S], F32)
            nc.sync.dma_start(out=c_sb, in_=ctab[:, :])
            nc.sync.dma_start(out=s_sb, in_=stab[:, :])
            mask_sb = const.tile([KT, 896], F32)
            nc.sync.dma_start(out=mask_sb, in_=masks[:, :])
            ones64 = const.tile([1, DK], F32)
            nc.vector.memset(ones64, 1.0)

            qT = qkpool.tile([FW, T], F32, tag="qT")
            kTt = qkpool.tile([FW, T], F32, tag="kT")
            v_sb = qkpool.tile([128, T // 128, 2 * DK + 2], F32, tag="v")
            outT = qkpool.tile([FW, T], F32, tag="outT")
            nc.vector.memset(v_sb, 1.0)  # bakes the ones columns

            # ---- QKV projections + RoPE, streaming x chunks ----
            for ci in range(NCH):
                t0 = ci * TCH
                sc = (ci % (S // TCH)) * TCH  # position within batch for rope tables
                xc = xch.tile([128, 8, TCH], F32, tag="x")
                nc.sync.dma_start(out=xc, in_=xT_r[:, :, t0 : t0 + TCH])

                for which, w_sb, dst in (("q", wq_sb, qT), ("k", wk_sb, kTt)):
                    ps = mmps.tile([FW, TCH], F32, tag="mm")
                    for dt in range(8):
                        nc.tensor.matmul(
                            ps,
                            w_sb[:, dt, :],
                            xc[:, dt, :],
                            start=(dt == 0),
                            stop=(dt == 7),
                        )
                    raw = tmp.tile([FW, TCH], F32, tag="raw")
                    nc.vector.tensor_copy(out=raw, in_=ps)
                    swp = tmp.tile([FW, TCH], F32, tag="swp")
                    # pair swap across partitions via two strided DMAs
                    nc.sync.dma_start(
                        out=swp[0 : FW - 1 : 2, :], in_=raw[1:FW:2, :]
                    )
                    nc.sync.dma_start(
                        out=swp[1:FW:2, :], in_=raw[0 : FW - 1 : 2, :]
                    )
                    dslice = dst[:, t0 : t0 + TCH]
                    nc.vector.tensor_mul(dslice, raw, c_sb[:, sc : sc + TCH])
                    t2 = tmp.tile([FW, TCH], F32, tag="ropetmp")
                    nc.vector.tensor_mul(t2, swp, s_sb[:, sc : sc + TCH])
                    nc.vector.tensor_add(dslice, dslice, t2)

                # V: [token, feature] layout, stationary = x chunk subtiles
                for sub in range(TCH // 128):
                    vps = mmps.tile([128, 128], F32, tag="mm")
                    for dt in range(8):
                        nc.tensor.matmul(
                            vps,
                            xc[:, dt, sub * 128 : (sub + 1) * 128],
                            wv_sb[:, dt, :],
                            start=(dt == 0),
                            stop=(dt == 7),
                        )
                    idx = t0 // 128 + sub
                    nc.vector.tensor_copy(out=v_sb[:, idx, 0:DK], in_=vps[:, 0:DK])
                    nc.vector.tensor_copy(
                        out=v_sb[:, idx, DK + 1 : 2 * DK + 1], in_=vps[:, DK : 2 * DK]
                    )

            # ---- causal attention, transposed-scores flash style ----
            for b in range(B):
                for qc in range(S // QC):
                    trow = b * S + qc * QC
                    for h2 in range(HPC):
                        fb = h2 * DK
                        vcol = h2 * (DK + 1)
                        pv = pvps.tile([DK + 1, QC], F32, tag="pv")
                        nkt = 4 * (qc + 1)
                        for kt in range(nkt):
                            kcol = b * S + kt * KT
                            st = stps.tile([KT, QC], F32, tag="st")
                            nc.tensor.matmul(
                                st,
                                kTt[fb : fb + DK, kcol : kcol + KT],
                                qT[fb : fb + DK, trow : trow + QC],
                                start=True,
                                stop=True,
                            )
                            pt = pts.tile([KT, QC], F32, tag="pt")
                            nc.scalar.activation(
                                out=pt, in_=st, func=mybir.ActivationFunctionType.Exp
                            )
                            if kt >= 4 * qc:
                                o = (kt - 4 * qc) * KT
                                nc.vector.tensor_mul(
                                    pt, pt, mask_sb[:, 384 - o : 384 - o + QC]
                                )
                            nc.tensor.matmul(
                                pv,
                                v_sb[:, b * (S // 128) + kt, vcol : vcol + DK + 1],
                                pt,
                                start=(kt == 0),
                                stop=(kt == nkt - 1),
                                skip_group_check=True,
                            )
                        linv = lpool.tile([1, QC], F32, tag="linv")
                        nc.vector.reciprocal(out=linv, in_=pv[DK : DK + 1, :])
                        # broadcast 1/l across the 64 head-dim partitions via a
                        # K=1 ones matmul (DMA/compute engines can't partition-bcast)
                        lbps = stps.tile([DK, QC], F32, tag="st")
                        nc.tensor.matmul(lbps, ones64, linv, start=True, stop=True)
                        lb = lpool.tile([DK, QC], F32, tag="lb")
                        nc.vector.tensor_copy(out=lb, in_=lbps)
                        nc.vector.tensor_mul(
                            outT[fb : fb + DK, trow : trow + QC], pv[0:DK, :], lb
                        )

            # ---- AllToAll: head-sharded -> token-sharded ----
            cc_in = dram.tile([N_CORES, FW, TSL], F32)
            cc_out = dram.tile([N_CORES, FW, TSL], F32)
            for p in range(N_CORES):
                nc.gpsimd.dma_start(
                    out=cc_in[p, :, :], in_=outT[:, p * TSL : (p + 1) * TSL]
                )
            nc.gpsimd.collective_compute(
                "AllToAll",
                mybir.AluOpType.bypass,
                replica_groups=[list(range(N_CORES))],
                ins=[cc_in[:].opt()],
                outs=[cc_out[:].opt()],
            )
            # reuses qT's slot (dead after attention) — Tile serializes via WAR deps
            orecv = qkpool.tile([128, N_CORES, TSL], F32, tag="qT")
            for p in range(N_CORES):
                nc.gpsimd.dma_start(out=orecv[:, p, :], in_=cc_out[p, :, :])

            # ---- output projection for this core's token slice ----
            wo_sb = []
            for p in range(N_CORES):
                wt = wopool.tile([128, D], F32, tag="wo")
                nc.sync.dma_start(out=wt, in_=woT[p * 128 : (p + 1) * 128, :])
                wo_sb.append(wt)
            for tt in range(TSL // 128):
                ysb = ypool.tile([128, D], F32, tag="y")
                for ec in range(D // 512):
                    yps = mmps.tile([128, 512], F32, tag="mm")
                    for p in range(N_CORES):
                        nc.tensor.matmul(
                            yps,
                            orecv[:, p, tt * 128 : (tt + 1) * 128],
                            wo_sb[p][:, ec * 512 : (ec + 1) * 512],
                            start=(p == 0),
                            stop=(p == N_CORES - 1),
                        )
                    nc.vector.tensor_copy(out=ysb[:, ec * 512 : (ec + 1) * 512], in_=yps)
                nc.sync.dma_start(out=y[tt * 128 : (tt + 1) * 128, :], in_=ysb)

    _spill_waits(nc)
    return nc


_NC_CACHE = None


def _get_nc():
    global _NC_CACHE
    if _NC_CACHE is None:
        _NC_CACHE = build_kernel()
    return _NC_CACHE


def _host_prep(x, Wq, Wk, Wv, Wo, token_positions):
    xT = np.ascontiguousarray(x.reshape(T, D).T)  # [D, T]
    WqT = np.ascontiguousarray(Wq.T) * np.float32(1.0 / math.sqrt(DK))
    WkT = np.ascontiguousarray(Wk.T)
    WvT = np.ascontiguousarray(Wv.T)
    WoT = np.ascontiguousarray(Wo.T)

    pos = token_positions.astype(np.float64)  # [S]
    i = (np.arange(FW) % DK) // 2  # pair index per row
    inv_freq = 1.0 / (10000.0 ** (2.0 * i / DK))  # [FW]
    ang = inv_freq[:, None] * pos[None, :]  # [FW, S]
    ctab = np.cos(ang).astype(np.float32)
    sgn = np.where(np.arange(FW) % 2 == 0, -1.0, 1.0)
    stab = (np.sin(ang) * sgn[:, None]).astype(np.float32)

    masks = (np.arange(896)[None, :] - 384 >= np.arange(KT)[:, None]).astype(
        np.float32
    )
    return xT, WqT, WkT, WvT, WoT, ctab, stab, masks


def kernel(x, Wq, Wk, Wv, Wo, mask, token_positions, num_heads, **run_kw):
    x = np.asarray(x)
    assert int(num_heads) == H and x.shape == (B, S, D)
    xT, WqT, WkT, WvT, WoT, ctab, stab, masks = _host_prep(
        np.asarray(x, np.float32),
        np.asarray(Wq, np.float32),
        np.asarray(Wk, np.float32),
        np.asarray(Wv, np.float32),
        np.asarray(Wo, np.float32),
        np.asarray(token_positions),
    )
    in_maps = []
    for c in range(N_CORES):
        cols = slice(c * FW, (c + 1) * FW)
        in_maps.append(
            {
                "xT": xT,
                "wq": np.ascontiguousarray(WqT[:, cols]),
                "wk": np.ascontiguousarray(WkT[:, cols]),
                "wv": np.ascontiguousarray(WvT[:, cols]),
                "woT": WoT,
                "ctab": ctab,
                "stab": stab,
                "masks": masks,
            }
        )
    nc = _get_nc()
    res = run_bass_kernel_spmd(
        nc, in_maps, core_ids=list(range(N_CORES)), **run_kw
    )
    yfull = np.concatenate([res.results[c]["y"] for c in range(N_CORES)], axis=0)
    out = yfull.reshape(B, S, D).astype(np.float32)
    kernel.last_results = res
    return out


# revision 20
# speedup vs baseline: 1.9376x; 1.9376x over previous
"""Multi-head causal attention with RoPE on 8 Trainium2 NeuronCores.

Problem: B=2, S=2048, D=1024, H=16 heads (dk=64), fp32, causal mask,
RoPE on Q/K, y = softmax(QK^T/sqrt(dk)) V projected by Wo.

Sharding: head-parallel. Core c owns 2 heads (columns c*128:(c+1)*128 of
the QKV projection output). Each core:
  1. computes Q^T,K^T,V for its heads from the full x (K-dim 1024 matmuls),
  2. applies RoPE in the transposed [head_dim, token] layout,
  3. runs causal attention with scores materialized transposed (ST[k,q]) so
     softmax needs no transposes: exp on ScalarE straight out of PSUM, the
     PV matmul consumes the exp'd tile as the moving operand, and a ones
     column appended to V makes the same matmul emit the softmax denominator,
  4. AllToAll (2 MB) flips head-sharded -> token-sharded,
  5. computes 1/8 of the output projection; host concatenates row slices.
A tiny dummy AllToAll at kernel start absorbs the one-time collective
warmup cost under the projection phase.
"""

import sys

for p in ("/opt/trn_rl_repo", "/root/.axon_site/_ro/trn_rl_repo"):
    if p not in sys.path:
        sys.path.insert(0, p)

import math

import numpy as np

import concourse.bass as bass
import concourse.tile as tile
from concourse import mybir
from concourse.bass_utils import run_bass_kernel_spmd

N_CORES = 8
B, S, D, H = 2, 2048, 1024, 16
DK = D // H          # 64
HPC = H // N_CORES   # heads per core = 2
FW = HPC * DK        # head-group width per core = 128
T = B * S            # 4096 flattened tokens
TCH = 512            # token chunk for projections
NCH = T // TCH       # 8 chunks
KT = 128             # k tile
QC = 512             # q chunk in attention
TSL = T // N_CORES   # 512 output rows per core

F32 = mybir.dt.float32
F32R = mybir.dt.float32r


def _spill_waits(nc, max_other=1):
    """walrus in this container allows 1 sync-wait per instruction; move
    excess waits onto preceding single-wait NoOps on the same engine."""
    n_new = 0
    for bb in nc.m.functions[0].blocks:
        newlist = []
        changed = False
        for inst in bb.instructions:
            si = inst.sync_info
            if si is not None and si.on_wait and len(si.on_wait) > max_other:
                waits = list(si.on_wait)
                overflow, keep = waits[:-max_other], waits[-max_other:]
                while overflow:
                    chunk, overflow = overflow[:1], overflow[1:]
                    nop = mybir.InstNoOp(
                        name=f"waitspill{n_new}-{inst.name}", ins=[], outs=[]
                    )
                    nop.engine = inst.engine
                    nop.debug = inst.debug
                    nop.sync_info = mybir.SyncInfo(on_wait=chunk, on_update=[])
                    newlist.append(nop)
                    n_new += 1
                si.on_wait = keep
                inst.sync_info = si
                changed = True
            newlist.append(inst)
        if changed:
            bb.instructions = newlist
    return n_new


def build_kernel():
    nc = bass.Bass("TRN2", num_devices=N_CORES)

    xT = nc.dram_tensor("xT", [D, T], F32R, kind="ExternalInput")
    wq = nc.dram_tensor("wq", [D, FW], F32R, kind="ExternalInput")  # pre-scaled 1/sqrt(dk)
    wk = nc.dram_tensor("wk", [D, FW], F32R, kind="ExternalInput")
    wv = nc.dram_tensor("wv", [D, FW], F32R, kind="ExternalInput")
    woT = nc.dram_tensor("woT", [D, D], F32R, kind="ExternalInput")
    ctab = nc.dram_tensor("ctab", [FW, S], F32, kind="ExternalInput")
    stab = nc.dram_tensor("stab", [FW, S], F32, kind="ExternalInput")
    masks = nc.dram_tensor("masks", [KT, 896], F32R, kind="ExternalInput")
    y = nc.dram_tensor("y", [TSL, D], F32, kind="ExternalOutput")

    xT_r = xT.rearrange("(dt p) t -> p dt t", p=128)  # [128, 8, T]

    with tile.TileContext(nc) as tc:
        with (
            tc.tile_pool(name="const", bufs=1) as const,
            tc.tile_pool(name="xch", bufs=2) as xch,
            tc.tile_pool(name="qk", bufs=1) as qkpool,
            tc.tile_pool(name="tmp", bufs=3) as tmp,
            tc.tile_pool(name="pts", bufs=6) as pts,
            tc.tile_pool(name="lpool", bufs=2) as lpool,
            tc.tile_pool(name="wo", bufs=8) as wopool,
            tc.tile_pool(name="yout", bufs=2) as ypool,
            tc.tile_pool(name="mm", bufs=2, space="PSUM") as mmps,
            tc.tile_pool(name="st", bufs=4, space="PSUM") as stps,
            tc.tile_pool(name="pv", bufs=2, space="PSUM") as pvps,
            tc.tile_pool(name="dram", bufs=1, space="DRAM") as dram,
        ):
            # ---- collective warmup (hidden under projection phase) ----
            warm_in = dram.tile([8, 16], F32)
            warm_out = dram.tile([8, 16], F32)
            wtile = const.tile([1, 128], F32)
            nc.vector.memset(wtile, 0.0)
            nc.gpsimd.dma_start(out=warm_in[:, :], in_=wtile[:1, :128].rearrange("p (a f) -> (p a) f", a=8))
            nc.gpsimd.collective_compute(
                "AllToAll",
                mybir.AluOpType.bypass,
                replica_groups=[list(range(N_CORES))],
                ins=[warm_in[:].opt()],
                outs=[warm_out[:].opt()],
            )

            # ---- constants ----
            wq_sb = const.tile([128, 8, FW], F32R)
            wk_sb = const.tile([128, 8, FW], F32R)
            wv_sb = const.tile([128, 8, FW], F32R)
            nc.sync.dma_start(out=wq_sb, in_=wq.rearrange("(dt p) f -> p dt f", p=128))
            nc.sync.dma_start(out=wk_sb, in_=wk.rearrange("(dt p) f -> p dt f", p=128))
            nc.sync.dma_start(out=wv_sb, in_=wv.rearrange("(dt p) f -> p dt f", p=128))
            c_sb = const.tile([FW, S], F32)
            s_sb = const.tile([FW, S], F32)
            nc.sync.dma_start(out=c_sb, in_=ctab[:, :])
            nc.sync.dma_start(out=s_sb, in_=stab[:, :])
            mask_sb = const.tile([KT, 896], F32R)
            nc.sync.dma_start(out=mask_sb, in_=masks[:, :])
            ones_f = const.tile([128, DK], F32)
            nc.vector.memset(ones_f, 1.0)
            ones64 = const.tile([1, DK], F32R)
            nc.vector.tensor_copy(out=ones64, in_=ones_f[:1, :])

            qT = qkpool.tile([FW, T], F32R, tag="qT")
            kTt = qkpool.tile([FW, T], F32R, tag="kT")
            v_sb = qkpool.tile([128, T // 128, 2 * DK + 2], F32R, tag="v")
            outT = qkpool.tile([FW, T], F32R, tag="outT")
            # bake the ones columns (f32r tiles can't be memset directly)
            vones = const.tile([128, T // 128], F32)
            nc.vector.memset(vones, 1.0)
            nc.vector.tensor_copy(out=v_sb[:, :, DK], in_=vones)
            nc.vector.tensor_copy(out=v_sb[:, :, 2 * DK + 1], in_=vones)

            # ---- QKV projections + RoPE (per x chunk) ----
            def do_qkv_chunk(ci):
                t0 = ci * TCH
                sc = (ci % (S // TCH)) * TCH  # position within batch for rope tables
                xc = xch.tile([128, 8, TCH], F32R, tag="x", name="xc")
                nc.sync.dma_start(out=xc, in_=xT_r[:, :, t0 : t0 + TCH])

                # Q accumulates in the "mm" pool, K in the "st" pool so the two
                # groups never stall on the same PSUM slot rotation.
                for which, w_sb, dst, pool, tag in (
                    ("q", wq_sb, qT, mmps, "mm"),
                    ("k", wk_sb, kTt, stps, "st"),
                ):
                    ps = pool.tile([FW, TCH], F32, tag=tag, name=f"{which}ps")
                    for dt in range(8):
                        nc.tensor.matmul(
                            ps,
                            w_sb[:, dt, :],
                            xc[:, dt, :],
                            start=(dt == 0),
                            stop=(dt == 7),
                        )
                    raw = tmp.tile([FW, TCH], F32R, tag="raw", name="raw")
                    nc.vector.tensor_copy(out=raw, in_=ps)
                    swp = tmp.tile([FW, TCH], F32R, tag="swp", name="swp")
                    # pair swap across partitions via two strided DMAs
                    nc.sync.dma_start(out=swp[0 : FW - 1 : 2, :], in_=raw[1:FW:2, :])
                    nc.sync.dma_start(out=swp[1:FW:2, :], in_=raw[0 : FW - 1 : 2, :])
                    dslice = dst[:, t0 : t0 + TCH]
                    nc.vector.tensor_mul(dslice, raw, c_sb[:, sc : sc + TCH])
                    t2 = tmp.tile([FW, TCH], F32R, tag="ropetmp", name="t2")
                    nc.gpsimd.tensor_mul(t2, swp, s_sb[:, sc : sc + TCH])
                    nc.vector.tensor_add(dslice, dslice, t2)

                # V: [token, feature] layout, stationary = x chunk subtiles
                for sub in range(TCH // 128):
                    pool, tag = ((mmps, "mm"), (stps, "st"))[sub % 2]
                    vps = pool.tile([128, 128], F32, tag=tag, name="vps")
                    for dt in range(8):
                        nc.tensor.matmul(
                            vps,
                            xc[:, dt, sub * 128 : (sub + 1) * 128],
                            wv_sb[:, dt, :],
                            start=(dt == 0),
                            stop=(dt == 7),
                        )
                    idx = t0 // 128 + sub
                    # ScalarE (idle during this phase) does the PSUM->SBUF copies
                    nc.scalar.copy(out=v_sb[:, idx, 0:DK], in_=vps[:, 0:DK])
                    nc.scalar.copy(
                        out=v_sb[:, idx, DK + 1 : 2 * DK + 1], in_=vps[:, DK : 2 * DK]
                    )

            # ---- causal attention for one batch ----
            # Transposed-scores flash style. The two heads' score matmuls run
            # concurrently in the PE array (row groups 0-63 / 64-127), and the
            # loop is software-pipelined one k-tile ahead: scores for kt+1 are
            # issued before the PV matmuls of kt, so the PE never waits on exp.
            def emit_st(b, qc, kt):
                trow = b * S + qc * QC
                kcol = b * S + kt * KT
                ptpair = []
                for h2 in range(HPC):
                    fb = h2 * DK
                    st = stps.tile([KT, QC], F32, tag="st", name=f"st{h2}")
                    nc.tensor.matmul(
                        st,
                        kTt[fb : fb + DK, kcol : kcol + KT],
                        qT[fb : fb + DK, trow : trow + QC],
                        start=True,
                        stop=True,
                    )
                    pt = pts.tile([KT, QC], F32R, tag="pt", name=f"pt{h2}")
                    nc.scalar.activation(
                        out=pt, in_=st, func=mybir.ActivationFunctionType.Exp
                    )
                    if kt >= 4 * qc:
                        o = (kt - 4 * qc) * KT
                        nc.gpsimd.tensor_mul(
                            pt, pt, mask_sb[:, 384 - o : 384 - o + QC]
                        )
                    ptpair.append(pt)
                return ptpair

            def emit_pv(b, qc, kt, pv2, ptpair):
                nkt = 4 * (qc + 1)
                for h2 in range(HPC):
                    vcol = h2 * (DK + 1)
                    nc.tensor.matmul(
                        pv2[h2],
                        v_sb[:, b * (S // 128) + kt, vcol : vcol + DK + 1],
                        ptpair[h2],
                        start=(kt == 0),
                        stop=(kt == nkt - 1),
                        skip_group_check=True,
                    )

            def do_attn_batch(b):
                for qc in range(S // QC):
                    trow = b * S + qc * QC
                    pv2 = [
                        pvps.tile([DK + 1, QC], F32, tag="pv", name=f"pv{h2}")
                        for h2 in range(HPC)
                    ]
                    nkt = 4 * (qc + 1)
                    prev = emit_st(b, qc, 0)
                    for kt in range(1, nkt):
                        cur = emit_st(b, qc, kt)
                        emit_pv(b, qc, kt - 1, pv2, prev)
                        prev = cur
                    emit_pv(b, qc, nkt - 1, pv2, prev)
                    for h2 in range(HPC):
                        fb = h2 * DK
                        pv = pv2[h2]
                        linv = lpool.tile([1, QC], F32R, tag="linv", name="linv")
                        with nc.allow_low_precision(reason="f32r rounding of 1/l"):
                            nc.vector.reciprocal(out=linv, in_=pv[DK : DK + 1, :])
                        # broadcast 1/l across the 64 head-dim partitions via a
                        # K=1 ones matmul (engines can't partition-broadcast)
                        lbps = stps.tile([DK, QC], F32, tag="st", name="lbps")
                        nc.tensor.matmul(lbps, ones64, linv, start=True, stop=True)
                        lb = lpool.tile([DK, QC], F32, tag="lb", name="lb")
                        nc.vector.tensor_copy(out=lb, in_=lbps)
                        nc.vector.tensor_mul(
                            outT[fb : fb + DK, trow : trow + QC], pv[0:DK, :], lb
                        )

            # batch 0 projections -> batch 0 attention (hides batch 1's x DMA)
            # -> batch 1 projections -> batch 1 attention
            for ci in range(NCH // 2):
                do_qkv_chunk(ci)
            do_attn_batch(0)
            for ci in range(NCH // 2, NCH):
                do_qkv_chunk(ci)
            do_attn_batch(1)

            # ---- AllToAll: head-sharded -> token-sharded ----
            cc_in = dram.tile([N_CORES, FW, TSL], F32R)
            cc_out = dram.tile([N_CORES, FW, TSL], F32R)
            for p in range(N_CORES):
                nc.gpsimd.dma_start(
                    out=cc_in[p, :, :], in_=outT[:, p * TSL : (p + 1) * TSL]
                )
            nc.gpsimd.collective_compute(
                "AllToAll",
                mybir.AluOpType.bypass,
                replica_groups=[list(range(N_CORES))],
                ins=[cc_in[:].opt()],
                outs=[cc_out[:].opt()],
            )
            # reuses qT's slot (dead after attention) — Tile serializes via WAR deps
            orecv = qkpool.tile([128, N_CORES, TSL], F32R, tag="qT")
            for p in range(N_CORES):
                nc.gpsimd.dma_start(out=orecv[:, p, :], in_=cc_out[p, :, :])

            # ---- output projection for this core's token slice ----
            wo_sb = []
            for p in range(N_CORES):
                wt = wopool.tile([128, D], F32R, tag="wo")
                nc.sync.dma_start(out=wt, in_=woT[p * 128 : (p + 1) * 128, :])
                wo_sb.append(wt)
            for tt in range(TSL // 128):
                ysb = ypool.tile([128, D], F32, tag="y")
                for ec in range(D // 512):
                    yps = mmps.tile([128, 512], F32, tag="mm")
                    for p in range(N_CORES):
                        nc.tensor.matmul(
                            yps,
                            orecv[:, p, tt * 128 : (tt + 1) * 128],
                            wo_sb[p][:, ec * 512 : (ec + 1) * 512],
                            start=(p == 0),
                            stop=(p == N_CORES - 1),
                        )
                    nc.vector.tensor_copy(out=ysb[:, ec * 512 : (ec + 1) * 512], in_=yps)
                nc.sync.dma_start(out=y[tt * 128 : (tt + 1) * 128, :], in_=ysb)

    _spill_waits(nc)
    return nc


_NC_CACHE = None


def _get_nc():
    global _NC_CACHE
    if _NC_CACHE is None:
        _NC_CACHE = build_kernel()
    return _NC_CACHE


def _host_prep(x, Wq, Wk, Wv, Wo, token_positions):
    xT = np.ascontiguousarray(x.reshape(T, D).T)  # [D, T]
    WqT = np.ascontiguousarray(Wq.T) * np.float32(1.0 / math.sqrt(DK))
    WkT = np.ascontiguousarray(Wk.T)
    WvT = np.ascontiguousarray(Wv.T)
    WoT = np.ascontiguousarray(Wo.T)

    pos = token_positions.astype(np.float64)  # [S]
    i = (np.arange(FW) % DK) // 2  # pair index per row
    inv_freq = 1.0 / (10000.0 ** (2.0 * i / DK))  # [FW]
    ang = inv_freq[:, None] * pos[None, :]  # [FW, S]
    ctab = np.cos(ang).astype(np.float32)
    sgn = np.where(np.arange(FW) % 2 == 0, -1.0, 1.0)
    stab = (np.sin(ang) * sgn[:, None]).astype(np.float32)

    masks = (np.arange(896)[None, :] - 384 >= np.arange(KT)[:, None]).astype(
        np.float32
    )
    return xT, WqT, WkT, WvT, WoT, ctab, stab, masks


def kernel(x, Wq, Wk, Wv, Wo, mask, token_positions, num_heads, **run_kw):
    x = np.asarray(x)
    assert int(num_heads) == H and x.shape == (B, S, D)
    xT, WqT, WkT, WvT, WoT, ctab, stab, masks = _host_prep(
        np.asarray(x, np.float32),
        np.asarray(Wq, np.float32),
        np.asarray(Wk, np.float32),
        np.asarray(Wv, np.float32),
        np.asarray(Wo, np.float32),
        np.asarray(token_positions),
    )
    in_maps = []
    for c in range(N_CORES):
        cols = slice(c * FW, (c + 1) * FW)
        in_maps.append(
            {
                "xT": xT,
                "wq": np.ascontiguousarray(WqT[:, cols]),
                "wk": np.ascontiguousarray(WkT[:, cols]),
                "wv": np.ascontiguousarray(WvT[:, cols]),
                "woT": WoT,
                "ctab": ctab,
                "stab": stab,
                "masks": masks,
            }
        )
    nc = _get_nc()
    res = run_bass_kernel_spmd(
        nc, in_maps, core_ids=list(range(N_CORES)), **run_kw
    )
    yfull = np.concatenate([res.results[c]["y"] for c in range(N_CORES)], axis=0)
    out = yfull.reshape(B, S, D).astype(np.float32)
    kernel.last_results = res
    return out
